# revision 23
# baseline (speedup 1.0000x reference)
"""NetVLAD-V2 Bass kernel for Trainium2, data-parallel over batch on 8 NeuronCores.

Math (per image):
    xn = x / ||x||_C                       (per-pixel L2 norm over channels)
    f  = W @ xn + b                        (1x1 conv, D=512)
    logits = Cn @ f                        (Cn = row-normalized centroids, K=64)
    s  = softmax_l(logits)                 (softmax over the 4096 pixels)
    vlad = s @ f^T                         (K, D)

Device-side refactoring (exact algebra):
    M  = Cn @ W   (K, C)  [host]
    logits[k,l] = (M @ x)[k,l] * inv_n[l] (+ Cn@b — cancels in softmax_l)
    vlad = diag(1/Z) @ (E' @ x^T) @ W^T + 1·b^T,  E' = exp(logits), Z = E'·1

Layout: logits computed TRANSPOSED (pixels on partitions) so the per-pixel
inv_n scale and the exp() fold into one ACT pass per chunk, and E'^T comes out
in exactly the layout the weighted-sum matmul needs.
"""

import os
import numpy as np

_CACHE: dict = {}

N, C, HW, D, K = 16, 128, 4096, 512, 64
NCORES = 8
NB = N // NCORES          # batch items per core
NCH = HW // 128           # 32 l-chunks of 128 pixels


def _build_program():
    from contextlib import ExitStack

    import concourse.bacc as bacc
    import concourse.mybir as mybir
    import concourse.tile as tile

    dt = mybir.dt
    f32, f32r, bf16 = dt.float32, dt.float32r, dt.bfloat16
    Act = mybir.ActivationFunctionType
    Alu = mybir.AluOpType

    nc = bacc.Bacc("TRN2", target_bir_lowering=False, debug=False)

    x_d = nc.dram_tensor("x", [NB, C, HW], f32, kind="ExternalInput").ap()
    mt_d = nc.dram_tensor("mt", [C, K], bf16, kind="ExternalInput").ap()
    wt_d = nc.dram_tensor("wt", [C, D], f32r, kind="ExternalInput").ap()
    idn_d = nc.dram_tensor("idn", [C, C], bf16, kind="ExternalInput").ap()
    id32_d = nc.dram_tensor("id32", [K, K], f32, kind="ExternalInput").ap()
    out_d = nc.dram_tensor("out", [NB, K, D], f32, kind="ExternalOutput").ap()

    with tile.TileContext(nc) as tc, ExitStack() as ctx:
        consts = ctx.enter_context(tc.tile_pool(name="consts", bufs=1))
        xpool = ctx.enter_context(tc.tile_pool(name="x", bufs=1))
        x2pool = ctx.enter_context(tc.tile_pool(name="x2", bufs=2))
        xtpool = ctx.enter_context(tc.tile_pool(name="xt", bufs=2))
        etpool = ctx.enter_context(tc.tile_pool(name="et", bufs=2))
        smallpool = ctx.enter_context(tc.tile_pool(name="small", bufs=2))
        outpool = ctx.enter_context(tc.tile_pool(name="outp", bufs=2))

        ps_xt = ctx.enter_context(tc.tile_pool(name="ps_xt", bufs=2, space="PSUM"))
        ps_lg = ctx.enter_context(tc.tile_pool(name="ps_lg", bufs=2, space="PSUM"))
        ps_ss = ctx.enter_context(tc.tile_pool(name="ps_ss", bufs=1, space="PSUM"))
        ps_az = ctx.enter_context(tc.tile_pool(name="ps_az", bufs=1, space="PSUM"))
        ps_fin = ctx.enter_context(tc.tile_pool(name="ps_fin", bufs=1, space="PSUM"))

        NACC = 2  # parallel A-accumulation chains (breaks exp->matmul serial chain)

        # --- constants (wt/id32 are only needed at the end; loaded later so
        # they don't delay the x load on the sync queue) ---
        mt_sb = consts.tile([C, K], bf16)
        nc.sync.dma_start(mt_sb[:], mt_d[:])
        idn_sb = consts.tile([C, C], bf16)
        nc.sync.dma_start(idn_sb[:], idn_d[:])
        ones_sb = consts.tile([C, 1], bf16)
        nc.vector.memset(ones_sb[:], 1.0)

        # --- phase 1: load x, squares, ss (per batch, pipelined) ---
        # Batch 0 is "primed": loaded as fp32 via HWDGE (starts much earlier
        # than the SWDGE path) and cast to bf16 on otherwise-idle ACT/DVE.
        # Batch 1 streams via SWDGE with the fp32->bf16 cast in the DMA.
        NPC = 4  # dma pieces per batch
        W_P = HW // NPC
        x_bfs = []
        ss_list = []
        x0_f32 = xpool.tile([C, HW], f32, tag="x0f")
        # one psum bank holds both batches' ss columns (each ss-matmul is a
        # closed accumulation group, so sharing the bank is safe)
        ss_all = ps_ss.tile([C, NB * NCH], f32, tag="ss")
        for n in range(NB):
            x_bf = xpool.tile([C, HW], bf16, tag=f"x{n}", name=f"x_bf{n}")
            x_bfs.append(x_bf)
            for t in range(NPC):
                sl = slice(t * W_P, (t + 1) * W_P)
                if n == 0:
                    nc.sync.dma_start(x0_f32[:, sl], x_d[n][:, sl])
                    if t % 2 == 0:
                        nc.scalar.copy(x_bf[:, sl], x0_f32[:, sl])
                    else:
                        nc.vector.tensor_copy(x_bf[:, sl], x0_f32[:, sl])
                else:
                    nc.gpsimd.dma_start(x_bf[:, sl], x_d[n][:, sl])
            ss_ps = ss_all[:, n * NCH:(n + 1) * NCH]
            ss_list.append(ss_ps)
            for t in range(NPC):
                x2 = x2pool.tile([C, W_P], bf16, tag="x2")
                if n == 0:
                    # square straight from the fp32 tile (SBUF-only -> 2x mode)
                    nc.vector.tensor_mul(
                        x2[:], x0_f32[:, t * W_P:(t + 1) * W_P],
                        x0_f32[:, t * W_P:(t + 1) * W_P]
                    )
                else:
                    nc.vector.tensor_mul(
                        x2[:], x_bf[:, t * W_P:(t + 1) * W_P],
                        x_bf[:, t * W_P:(t + 1) * W_P]
                    )
                for jj in range(W_P // 128):
                    j = t * (W_P // 128) + jj
                    nc.tensor.matmul(
                        ss_ps[:, j:j + 1],
                        lhsT=x2[:, jj * 128:(jj + 1) * 128],
                        rhs=ones_sb[:],
                        start=True, stop=True,
                    )

        wt_sb = consts.tile([C, D], f32r)
        nc.sync.dma_start(wt_sb[:], wt_d[:])
        id32_sb = consts.tile([K, K], f32)
        nc.sync.dma_start(id32_sb[:], id32_d[:])

        for n in range(NB):
            x_bf = x_bfs[n]
            ss_ps = ss_list[n]
            col = lambda t, j: t[:, j:j + 1]  # noqa: E731

            # inv_n = exp(-0.5 ln ss); nrm = exp(0.5 ln ss) = ||x_l||
            ln_t = smallpool.tile([C, NCH], f32, tag="ln", name=f"ln{n}")
            nc.scalar.activation(ln_t[:], ss_ps[:], Act.Ln)
            lninv = smallpool.tile([C, NCH], f32, tag="lninv", name=f"lninv{n}")
            nc.vector.tensor_scalar_mul(lninv[:], ln_t[:], -0.5)
            inv_n = smallpool.tile([C, NCH], f32, tag="invn", name=f"invn{n}")
            nc.scalar.activation(inv_n[:], lninv[:], Act.Exp)
            nrm_bf = smallpool.tile([C, NCH], bf16, tag="nrm", name=f"nrm{n}")
            nc.scalar.activation(nrm_bf[:], ln_t[:], Act.Exp, scale=0.5)

            # --- per-chunk: transpose x, logits^T, exp ---
            # xT layout: (128, NCH, 132): cols 0..127 = x^T chunk, col 128 = n[l]
            # (col 128 recovers Z: sum_l (e*inv_n)[k,l] * n[l] = sum_l e[k,l])
            xt_sb = xtpool.tile([C, NCH, 132], bf16, tag="xt")
            nc.vector.tensor_copy(xt_sb[:, :, 128], nrm_bf[:])
            # E'^T layout: (128, NCH, 66): cols 0..63 = exp chunk, col 64 = ones
            et_sb = etpool.tile([C, NCH, 66], bf16, tag="et")
            nc.vector.memset(et_sb[:, :, 64:65], 1.0)

            GX = 4   # xT chunks per psum group (1 bank)
            GL = 8   # logitsT chunks per psum group (1 bank)
            for g in range(NCH // GX):
                xt_ps = ps_xt.tile([C, GX * 128], bf16, tag="xt_ps")
                for jj in range(GX):
                    j = g * GX + jj
                    nc.tensor.transpose(
                        xt_ps[:, jj * 128:(jj + 1) * 128],
                        x_bf[:, j * 128:(j + 1) * 128],
                        idn_sb[:],
                    )
                # PSUM(bf16) -> SBUF(bf16) copy, 2x mode
                nc.vector.tensor_copy(
                    xt_sb[:, g * GX:(g + 1) * GX, 0:128], xt_ps[:]
                )
            for g in range(NCH // GL):
                lg_ps = ps_lg.tile([C, GL * K], f32, tag="lg_ps")
                for jj in range(GL):
                    j = g * GL + jj
                    nc.tensor.matmul(
                        lg_ps[:, jj * K:(jj + 1) * K],
                        lhsT=x_bf[:, j * 128:(j + 1) * 128],
                        rhs=mt_sb[:],
                        start=True, stop=True,
                    )
                for jj in range(GL):
                    j = g * GL + jj
                    # E'^T = exp(a*inv_n - 0.5 ln ss) = exp(a*inv_n) * inv_n
                    nc.scalar.activation(
                        et_sb[:, j, 0:K],
                        lg_ps[:, jj * K:(jj + 1) * K],
                        Act.Exp,
                        scale=col(inv_n, j),
                        bias=col(lninv, j),
                    )

            # --- A^T and Z via NACC parallel accumulated matmul chains ---
            # out (65,129): rows 0..63 = A(k,c) cols 0..127, col 128 = Z[k]
            az_parts = [
                ps_az.tile([65, 129], f32, tag=f"az{a}", name=f"az{a}_{n}")
                for a in range(NACC)
            ]
            SPAN = NCH // NACC
            for a in range(NACC):
                for jj in range(SPAN):
                    j = a * SPAN + jj
                    nc.tensor.matmul(
                        az_parts[a][:],
                        lhsT=et_sb[:, j, 0:65],
                        rhs=xt_sb[:, j, 0:129],
                        start=(jj == 0), stop=(jj == SPAN - 1),
                    )
            # combine the partial accumulators (one PSUM operand per DVE op,
            # and never in-place)
            prev = None
            for a in range(NACC):
                nxt = smallpool.tile([65, 129], f32, tag=f"azc{a}", name=f"azc{a}_{n}")
                if prev is None:
                    nc.vector.tensor_copy(nxt[:], az_parts[a][:])
                else:
                    nc.vector.tensor_add(nxt[:], az_parts[a][:], prev[:])
                prev = nxt
            az_sb = prev

            # --- finalize ---
            rz = smallpool.tile([K, 1], f32, tag="rz")
            nc.vector.reciprocal(rz[:], az_sb[0:K, 128:129])
            at_ps = ps_fin.tile([C, K], f32, tag="fin")
            nc.tensor.transpose(at_ps[:], az_sb[0:K, 0:128], id32_sb[:])
            at_sb = smallpool.tile([C, K], f32r, tag="at_sb")
            nc.vector.tensor_copy(at_sb[:], at_ps[:])
            vl_ps = ps_fin.tile([K, D], f32, tag="fin")
            nc.tensor.matmul(
                vl_ps[:],
                lhsT=at_sb[:],
                rhs=wt_sb[:],
                start=True, stop=True,
            )
            vl_sb = outpool.tile([K, D], f32, tag="vl_sb")
            nc.vector.tensor_scalar(
                out=vl_sb[:], in0=vl_ps[:], scalar1=rz[:], scalar2=None, op0=Alu.mult
            )
            nc.sync.dma_start(out_d[n], vl_sb[:])

    nc.compile()
    return nc


def _get_program():
    if "nc" not in _CACHE:
        _CACHE["nc"] = _build_program()
    return _CACHE["nc"]


def _host_prep(conv_w, conv_b, centroids):
    import ml_dtypes

    cn = centroids / np.maximum(
        np.sqrt((centroids * centroids).sum(1, keepdims=True)), 1e-12
    )
    m = cn @ conv_w                                   # (K, C)
    mt = np.ascontiguousarray(m.T).astype(ml_dtypes.bfloat16)      # (C, K)
    wt = np.ascontiguousarray(conv_w.T).astype(np.float32)         # (C, D)
    idn = np.eye(C, dtype=ml_dtypes.bfloat16)
    id32 = np.eye(K, dtype=np.float32)
    return mt, wt, idn, id32


def _install_ntff_hook():
    """The image's antenv package lacks axon_hooks, so boot() skipped NTFF
    profiling setup. Recreate the module + install the ctypes hook so
    trace=True yields per-core exec times."""
    import sys as _sys
    import types as _types

    if "antenv.axon_hooks" in _sys.modules:
        return
    try:
        from trn_agent_boot.trn_boot import _ntff_profile_via_ctypes
        hook = _ntff_profile_via_ctypes("/opt/axon/libaxon_pjrt.so")
    except Exception:
        hook = None
    mod = _types.ModuleType("antenv.axon_hooks")
    mod._hook = hook
    mod.get_axon_ntff_profile_hook = lambda: mod._hook
    def _set(h):
        mod._hook = h
    mod.set_axon_ntff_profile_hook = _set
    _sys.modules["antenv.axon_hooks"] = mod


def kernel(x, conv_w, conv_b, centroids):
    _install_ntff_hook()
    from concourse.bass_utils import run_bass_kernel_spmd

    x = np.asarray(x, dtype=np.float32)
    conv_w = np.asarray(conv_w, dtype=np.float32)
    conv_b = np.asarray(conv_b, dtype=np.float32)
    centroids = np.asarray(centroids, dtype=np.float32)

    nc = _get_program()
    mt, wt, idn, id32 = _host_prep(conv_w, conv_b, centroids)

    xr = x.reshape(N, C, HW)
    in_maps = []
    for c in range(NCORES):
        in_maps.append({
            "x": np.ascontiguousarray(xr[c * NB:(c + 1) * NB]),
            "mt": mt, "wt": wt, "idn": idn, "id32": id32,
        })

    trace = bool(int(os.environ.get("NETVLAD_TRACE", "0")))
    res = run_bass_kernel_spmd(nc, in_maps, core_ids=list(range(NCORES)), trace=trace)
    _CACHE["last_result"] = res

    out = np.concatenate([r["out"] for r in res.results], axis=0)  # (16, K, D)
    if np.any(conv_b):
        out = out + conv_b[None, None, :]
    return out.astype(np.float32)


# revision 25
# speedup vs baseline: 1.1218x; 1.1218x over previous
"""NetVLAD-V2 Bass kernel for Trainium2, data-parallel over batch on 8 NeuronCores.

Math (per image):
    xn = x / ||x||_C                       (per-pixel L2 norm over channels)
    f  = W @ xn + b                        (1x1 conv, D=512)
    logits = Cn @ f                        (Cn = row-normalized centroids, K=64)
    s  = softmax_l(logits)                 (softmax over the 4096 pixels)
    vlad = s @ f^T                         (K, D)

Device-side refactoring (exact algebra):
    M  = Cn @ W   (K, C)  [host]
    logits[k,l] = (M @ x)[k,l] * inv_n[l] (+ Cn@b — cancels in softmax_l)
    vlad = diag(1/Z) @ (E' @ x^T) @ W^T + 1·b^T,  E' = exp(logits), Z = E'·1

Layout: logits computed TRANSPOSED (pixels on partitions) so the per-pixel
inv_n scale and the exp() fold into one ACT pass per chunk, and E'^T comes out
in exactly the layout the weighted-sum matmul needs.
"""

import os
import numpy as np

_CACHE: dict = {}

N, C, HW, D, K = 16, 128, 4096, 512, 64
NCORES = 8
NB = N // NCORES          # batch items per core
NCH = HW // 128           # 32 l-chunks of 128 pixels


def _build_program():
    from contextlib import ExitStack

    import concourse.bacc as bacc
    import concourse.mybir as mybir
    import concourse.tile as tile

    dt = mybir.dt
    f32, f32r, bf16 = dt.float32, dt.float32r, dt.bfloat16
    Act = mybir.ActivationFunctionType
    Alu = mybir.AluOpType

    # The activation-table chooser greedily picks the first set containing
    # each function, which makes Exp/Ln ping-pong between two sets (one
    # ~1.5us ACT_TABLE_LOAD per swap). Mask Exp/Ln out of every set except
    # the combined one (positions preserved, so set IDs stay valid) so the
    # whole kernel uses a single table load.
    if not getattr(bacc, "_netvlad_act_patch", False):
        _orig_get_tables = bacc.get_activation_tables

        def _patched_get_tables(arch):
            tabs = _orig_get_tables(arch)
            both = {Act.Exp, Act.Ln}
            out = {}
            for name, funcs in tabs.items():
                if name == "natural_log_exp_and_others":
                    out[name] = funcs
                else:
                    out[name] = funcs - both
            return out

        bacc.get_activation_tables = _patched_get_tables
        bacc._netvlad_act_patch = True

    nc = bacc.Bacc("TRN2", target_bir_lowering=False, debug=False)

    x_d = nc.dram_tensor("x", [NB, C, HW], f32, kind="ExternalInput").ap()
    mt_d = nc.dram_tensor("mt", [C, K], bf16, kind="ExternalInput").ap()
    wt_d = nc.dram_tensor("wt", [C, D], f32r, kind="ExternalInput").ap()
    idn_d = nc.dram_tensor("idn", [C, C], bf16, kind="ExternalInput").ap()
    id32_d = nc.dram_tensor("id32", [K, K], f32, kind="ExternalInput").ap()
    out_d = nc.dram_tensor("out", [NB, K, D], f32, kind="ExternalOutput").ap()

    with tile.TileContext(nc) as tc, ExitStack() as ctx:
        consts = ctx.enter_context(tc.tile_pool(name="consts", bufs=1))
        xpool = ctx.enter_context(tc.tile_pool(name="x", bufs=1))
        x2pool = ctx.enter_context(tc.tile_pool(name="x2", bufs=2))
        xtpool = ctx.enter_context(tc.tile_pool(name="xt", bufs=2))
        etpool = ctx.enter_context(tc.tile_pool(name="et", bufs=2))
        smallpool = ctx.enter_context(tc.tile_pool(name="small", bufs=2))
        outpool = ctx.enter_context(tc.tile_pool(name="outp", bufs=2))

        ps_xt = ctx.enter_context(tc.tile_pool(name="ps_xt", bufs=2, space="PSUM"))
        ps_lg = ctx.enter_context(tc.tile_pool(name="ps_lg", bufs=2, space="PSUM"))
        ps_ss = ctx.enter_context(tc.tile_pool(name="ps_ss", bufs=1, space="PSUM"))
        ps_az = ctx.enter_context(tc.tile_pool(name="ps_az", bufs=1, space="PSUM"))
        ps_fin = ctx.enter_context(tc.tile_pool(name="ps_fin", bufs=1, space="PSUM"))

        NACC = 2  # parallel A-accumulation chains (breaks exp->matmul serial chain)

        # --- constants (wt/id32 are only needed at the end; loaded later so
        # they don't delay the x load on the sync queue) ---
        mt_sb = consts.tile([C, K], bf16)
        nc.sync.dma_start(mt_sb[:], mt_d[:])
        idn_sb = consts.tile([C, C], bf16)
        nc.sync.dma_start(idn_sb[:], idn_d[:])
        ones_sb = consts.tile([C, 1], bf16)
        nc.vector.memset(ones_sb[:], 1.0)

        # --- phase 1: load x, squares, ss (per batch, pipelined) ---
        # Batch 0 is "primed": loaded as fp32 via HWDGE (starts much earlier
        # than the SWDGE path) and cast to bf16 on otherwise-idle ACT/DVE.
        # Batch 1 streams via SWDGE with the fp32->bf16 cast in the DMA.
        NPC = 4  # dma pieces per batch
        W_P = HW // NPC
        x_bfs = []
        ss_list = []
        # one psum bank holds both batches' ss columns (each ss-matmul is a
        # closed accumulation group, so sharing the bank is safe)
        ss_all = ps_ss.tile([C, NB * NCH], f32, tag="ss")
        for n in range(NB):
            x_bf = xpool.tile([C, HW], bf16, tag=f"x{n}", name=f"x_bf{n}")
            x_bfs.append(x_bf)
            for t in range(NPC):
                sl = slice(t * W_P, (t + 1) * W_P)
                nc.gpsimd.dma_start(x_bf[:, sl], x_d[n][:, sl])
            ss_ps = ss_all[:, n * NCH:(n + 1) * NCH]
            ss_list.append(ss_ps)
            for t in range(NPC):
                x2 = x2pool.tile([C, W_P], bf16, tag="x2")
                nc.vector.tensor_mul(
                    x2[:], x_bf[:, t * W_P:(t + 1) * W_P],
                    x_bf[:, t * W_P:(t + 1) * W_P]
                )
                for jj in range(W_P // 128):
                    j = t * (W_P // 128) + jj
                    nc.tensor.matmul(
                        ss_ps[:, j:j + 1],
                        lhsT=x2[:, jj * 128:(jj + 1) * 128],
                        rhs=ones_sb[:],
                        start=True, stop=True,
                    )

        wt_sb = consts.tile([C, D], f32r)
        nc.sync.dma_start(wt_sb[:], wt_d[:])
        id32_sb = consts.tile([K, K], f32)
        nc.sync.dma_start(id32_sb[:], id32_d[:])

        for n in range(NB):
            x_bf = x_bfs[n]
            ss_ps = ss_list[n]
            col = lambda t, j: t[:, j:j + 1]  # noqa: E731

            # inv_n = exp(-0.5 ln ss); nrm = exp(0.5 ln ss) = ||x_l||
            ln_t = smallpool.tile([C, NCH], f32, tag="ln", name=f"ln{n}")
            nc.scalar.activation(ln_t[:], ss_ps[:], Act.Ln)
            lninv = smallpool.tile([C, NCH], f32, tag="lninv", name=f"lninv{n}")
            nc.vector.tensor_scalar_mul(lninv[:], ln_t[:], -0.5)
            inv_n = smallpool.tile([C, NCH], f32, tag="invn", name=f"invn{n}")
            nc.scalar.activation(inv_n[:], lninv[:], Act.Exp)
            nrm_bf = smallpool.tile([C, NCH], bf16, tag="nrm", name=f"nrm{n}")
            nc.scalar.activation(nrm_bf[:], ln_t[:], Act.Exp, scale=0.5)

            # --- per-chunk: transpose x, logits^T, exp ---
            # xT layout: (128, NCH, 132): cols 0..127 = x^T chunk, col 128 = n[l]
            # (col 128 recovers Z: sum_l (e*inv_n)[k,l] * n[l] = sum_l e[k,l])
            xt_sb = xtpool.tile([C, NCH, 132], bf16, tag="xt")
            nc.vector.tensor_copy(xt_sb[:, :, 128], nrm_bf[:])
            # E'^T layout: (128, NCH, 66): cols 0..63 = exp chunk, col 64 = ones
            et_sb = etpool.tile([C, NCH, 66], bf16, tag="et")
            nc.vector.memset(et_sb[:, :, 64:65], 1.0)

            GX = 4   # xT chunks per psum group (1 bank)
            GL = 8   # logitsT chunks per psum group (1 bank)
            for g in range(NCH // GX):
                xt_ps = ps_xt.tile([C, GX * 128], bf16, tag="xt_ps")
                for jj in range(GX):
                    j = g * GX + jj
                    nc.tensor.transpose(
                        xt_ps[:, jj * 128:(jj + 1) * 128],
                        x_bf[:, j * 128:(j + 1) * 128],
                        idn_sb[:],
                    )
                # PSUM(bf16) -> SBUF(bf16) copy, 2x mode
                nc.vector.tensor_copy(
                    xt_sb[:, g * GX:(g + 1) * GX, 0:128], xt_ps[:]
                )
            for g in range(NCH // GL):
                lg_ps = ps_lg.tile([C, GL * K], f32, tag="lg_ps")
                for jj in range(GL):
                    j = g * GL + jj
                    nc.tensor.matmul(
                        lg_ps[:, jj * K:(jj + 1) * K],
                        lhsT=x_bf[:, j * 128:(j + 1) * 128],
                        rhs=mt_sb[:],
                        start=True, stop=True,
                    )
                for jj in range(GL):
                    j = g * GL + jj
                    # E'^T = exp(a*inv_n - 0.5 ln ss) = exp(a*inv_n) * inv_n
                    nc.scalar.activation(
                        et_sb[:, j, 0:K],
                        lg_ps[:, jj * K:(jj + 1) * K],
                        Act.Exp,
                        scale=col(inv_n, j),
                        bias=col(lninv, j),
                    )

            # --- A^T and Z via NACC parallel accumulated matmul chains ---
            # out (65,129): rows 0..63 = A(k,c) cols 0..127, col 128 = Z[k]
            az_parts = [
                ps_az.tile([65, 129], f32, tag=f"az{a}", name=f"az{a}_{n}")
                for a in range(NACC)
            ]
            SPAN = NCH // NACC
            for a in range(NACC):
                for jj in range(SPAN):
                    j = a * SPAN + jj
                    nc.tensor.matmul(
                        az_parts[a][:],
                        lhsT=et_sb[:, j, 0:65],
                        rhs=xt_sb[:, j, 0:129],
                        start=(jj == 0), stop=(jj == SPAN - 1),
                    )
            # combine the partial accumulators (one PSUM operand per DVE op,
            # and never in-place)
            prev = None
            for a in range(NACC):
                nxt = smallpool.tile([65, 129], f32, tag=f"azc{a}", name=f"azc{a}_{n}")
                if prev is None:
                    nc.vector.tensor_copy(nxt[:], az_parts[a][:])
                else:
                    nc.vector.tensor_add(nxt[:], az_parts[a][:], prev[:])
                prev = nxt
            az_sb = prev

            # --- finalize ---
            rz = smallpool.tile([K, 1], f32, tag="rz")
            nc.vector.reciprocal(rz[:], az_sb[0:K, 128:129])
            at_ps = ps_fin.tile([C, K], f32, tag="fin")
            nc.tensor.transpose(at_ps[:], az_sb[0:K, 0:128], id32_sb[:])
            at_sb = smallpool.tile([C, K], f32r, tag="at_sb")
            nc.vector.tensor_copy(at_sb[:], at_ps[:])
            vl_ps = ps_fin.tile([K, D], f32, tag="fin")
            nc.tensor.matmul(
                vl_ps[:],
                lhsT=at_sb[:],
                rhs=wt_sb[:],
                start=True, stop=True,
            )
            vl_sb = outpool.tile([K, D], f32, tag="vl_sb")
            nc.vector.tensor_scalar(
                out=vl_sb[:], in0=vl_ps[:], scalar1=rz[:], scalar2=None, op0=Alu.mult
            )
            nc.sync.dma_start(out_d[n], vl_sb[:])

    nc.compile()
    return nc


def _get_program():
    if "nc" not in _CACHE:
        _CACHE["nc"] = _build_program()
    return _CACHE["nc"]


def _host_prep(conv_w, conv_b, centroids):
    import ml_dtypes

    cn = centroids / np.maximum(
        np.sqrt((centroids * centroids).sum(1, keepdims=True)), 1e-12
    )
    m = cn @ conv_w                                   # (K, C)
    mt = np.ascontiguousarray(m.T).astype(ml_dtypes.bfloat16)      # (C, K)
    wt = np.ascontiguousarray(conv_w.T).astype(np.float32)         # (C, D)
    idn = np.eye(C, dtype=ml_dtypes.bfloat16)
    id32 = np.eye(K, dtype=np.float32)
    return mt, wt, idn, id32


def _install_ntff_hook():
    """The image's antenv package lacks axon_hooks, so boot() skipped NTFF
    profiling setup. Recreate the module + install the ctypes hook so
    trace=True yields per-core exec times."""
    import sys as _sys
    import types as _types

    if "antenv.axon_hooks" in _sys.modules:
        return
    try:
        from trn_agent_boot.trn_boot import _ntff_profile_via_ctypes
        hook = _ntff_profile_via_ctypes("/opt/axon/libaxon_pjrt.so")
    except Exception:
        hook = None
    mod = _types.ModuleType("antenv.axon_hooks")
    mod._hook = hook
    mod.get_axon_ntff_profile_hook = lambda: mod._hook
    def _set(h):
        mod._hook = h
    mod.set_axon_ntff_profile_hook = _set
    _sys.modules["antenv.axon_hooks"] = mod


def kernel(x, conv_w, conv_b, centroids):
    _install_ntff_hook()
    from concourse.bass_utils import run_bass_kernel_spmd

    x = np.asarray(x, dtype=np.float32)
    conv_w = np.asarray(conv_w, dtype=np.float32)
    conv_b = np.asarray(conv_b, dtype=np.float32)
    centroids = np.asarray(centroids, dtype=np.float32)

    nc = _get_program()
    mt, wt, idn, id32 = _host_prep(conv_w, conv_b, centroids)

    xr = x.reshape(N, C, HW)
    in_maps = []
    for c in range(NCORES):
        in_maps.append({
            "x": np.ascontiguousarray(xr[c * NB:(c + 1) * NB]),
            "mt": mt, "wt": wt, "idn": idn, "id32": id32,
        })

    trace = bool(int(os.environ.get("NETVLAD_TRACE", "0")))
    res = run_bass_kernel_spmd(nc, in_maps, core_ids=list(range(NCORES)), trace=trace)
    _CACHE["last_result"] = res

    out = np.concatenate([r["out"] for r in res.results], axis=0)  # (16, K, D)
    if np.any(conv_b):
        out = out + conv_b[None, None, :]
    return out.astype(np.float32)


# revision 26
# speedup vs baseline: 1.3864x; 1.2359x over previous
"""NetVLAD-V2 Bass kernel for Trainium2, data-parallel over batch on 8 NeuronCores.

Math (per image):
    xn = x / ||x||_C                       (per-pixel L2 norm over channels)
    f  = W @ xn + b                        (1x1 conv, D=512)
    logits = Cn @ f                        (Cn = row-normalized centroids, K=64)
    s  = softmax_l(logits)                 (softmax over the 4096 pixels)
    vlad = s @ f^T                         (K, D)

Device-side refactoring (exact algebra):
    M  = Cn @ W   (K, C)  [host]
    logits[k,l] = (M @ x)[k,l] * inv_n[l] (+ Cn@b — cancels in softmax_l)
    vlad = diag(1/Z) @ (E' @ x^T) @ W^T + 1·b^T,  E' = exp(logits), Z = E'·1

Layout: logits computed TRANSPOSED (pixels on partitions) so the per-pixel
inv_n scale and the exp() fold into one ACT pass per chunk, and E'^T comes out
in exactly the layout the weighted-sum matmul needs.
"""

import os
import numpy as np

_CACHE: dict = {}

N, C, HW, D, K = 16, 128, 4096, 512, 64
NCORES = 8
NB = N // NCORES          # batch items per core
NCH = HW // 128           # 32 l-chunks of 128 pixels


def _build_program():
    from contextlib import ExitStack

    import concourse.bacc as bacc
    import concourse.mybir as mybir
    import concourse.tile as tile

    dt = mybir.dt
    f32, f32r, bf16 = dt.float32, dt.float32r, dt.bfloat16
    Act = mybir.ActivationFunctionType
    Alu = mybir.AluOpType

    # The activation-table chooser greedily picks the first set containing
    # each function, which makes Exp/Ln ping-pong between two sets (one
    # ~1.5us ACT_TABLE_LOAD per swap). Mask Exp/Ln out of every set except
    # the combined one (positions preserved, so set IDs stay valid) so the
    # whole kernel uses a single table load.
    if not getattr(bacc, "_netvlad_act_patch", False):
        _orig_get_tables = bacc.get_activation_tables

        def _patched_get_tables(arch):
            tabs = _orig_get_tables(arch)
            both = {Act.Exp, Act.Ln}
            out = {}
            for name, funcs in tabs.items():
                if name == "natural_log_exp_and_others":
                    out[name] = funcs
                else:
                    out[name] = funcs - both
            return out

        bacc.get_activation_tables = _patched_get_tables
        bacc._netvlad_act_patch = True

    nc = bacc.Bacc("TRN2", target_bir_lowering=False, debug=False)

    x_d = nc.dram_tensor("x", [NB, C, HW], f32, kind="ExternalInput").ap()
    mt_d = nc.dram_tensor("mt", [C, K], bf16, kind="ExternalInput").ap()
    wt_d = nc.dram_tensor("wt", [C, D], f32r, kind="ExternalInput").ap()
    idn_d = nc.dram_tensor("idn", [C, C], bf16, kind="ExternalInput").ap()
    id32_d = nc.dram_tensor("id32", [K, K], f32, kind="ExternalInput").ap()
    out_d = nc.dram_tensor("out", [NB, K, D], f32, kind="ExternalOutput").ap()

    with tile.TileContext(nc) as tc, ExitStack() as ctx:
        consts = ctx.enter_context(tc.tile_pool(name="consts", bufs=1))
        xpool = ctx.enter_context(tc.tile_pool(name="x", bufs=1))
        x2pool = ctx.enter_context(tc.tile_pool(name="x2", bufs=2))
        xtpool = ctx.enter_context(tc.tile_pool(name="xt", bufs=2))
        etpool = ctx.enter_context(tc.tile_pool(name="et", bufs=2))
        smallpool = ctx.enter_context(tc.tile_pool(name="small", bufs=2))
        outpool = ctx.enter_context(tc.tile_pool(name="outp", bufs=2))

        ps_xt = ctx.enter_context(tc.tile_pool(name="ps_xt", bufs=2, space="PSUM"))
        ps_lg = ctx.enter_context(tc.tile_pool(name="ps_lg", bufs=2, space="PSUM"))
        ps_ss = ctx.enter_context(tc.tile_pool(name="ps_ss", bufs=1, space="PSUM"))
        ps_az = ctx.enter_context(tc.tile_pool(name="ps_az", bufs=1, space="PSUM"))
        ps_fin = ctx.enter_context(tc.tile_pool(name="ps_fin", bufs=1, space="PSUM"))

        NACC = 1  # parallel A-accumulation chains (breaks exp->matmul serial chain)

        # --- constants (wt/id32 are only needed at the end; loaded later so
        # they don't delay the x load on the sync queue) ---
        mt_sb = consts.tile([C, K], bf16)
        nc.sync.dma_start(mt_sb[:], mt_d[:])
        wt_sb = consts.tile([C, D], f32r)
        nc.sync.dma_start(wt_sb[:], wt_d[:])
        idn_sb = consts.tile([C, C], bf16)
        nc.sync.dma_start(idn_sb[:], idn_d[:])
        id32_sb = consts.tile([K, K], f32)
        nc.sync.dma_start(id32_sb[:], id32_d[:])
        ones_sb = consts.tile([C, 1], bf16)
        nc.vector.memset(ones_sb[:], 1.0)

        # --- phase 1: load x, squares, ss (per batch, pipelined) ---
        # Batch 0 is "primed": loaded as fp32 via HWDGE (starts much earlier
        # than the SWDGE path) and cast to bf16 on otherwise-idle ACT/DVE.
        # Batch 1 streams via SWDGE with the fp32->bf16 cast in the DMA.
        NPC = 4  # dma pieces per batch
        W_P = HW // NPC
        x_bfs = []
        ss_list = []
        for n in range(NB):
            x_bf = xpool.tile([C, HW], bf16, tag=f"x{n}", name=f"x_bf{n}")
            x_bfs.append(x_bf)
            for t in range(NPC):
                sl = slice(t * W_P, (t + 1) * W_P)
                nc.gpsimd.dma_start(x_bf[:, sl], x_d[n][:, sl])
            ss_ps = ps_ss.tile([C, NCH], f32, tag="ss", name=f"ss{n}")
            ss_list.append(ss_ps)
            for t in range(NPC):
                x2 = x2pool.tile([C, W_P], bf16, tag="x2")
                nc.vector.tensor_mul(
                    x2[:], x_bf[:, t * W_P:(t + 1) * W_P],
                    x_bf[:, t * W_P:(t + 1) * W_P]
                )
                for jj in range(W_P // 128):
                    j = t * (W_P // 128) + jj
                    nc.tensor.matmul(
                        ss_ps[:, j:j + 1],
                        lhsT=x2[:, jj * 128:(jj + 1) * 128],
                        rhs=ones_sb[:],
                        start=True, stop=True,
                    )

        for n in range(NB):
            x_bf = x_bfs[n]
            ss_ps = ss_list[n]
            col = lambda t, j: t[:, j:j + 1]  # noqa: E731

            # inv_n = exp(-0.5 ln ss); nrm = exp(0.5 ln ss) = ||x_l||
            ln_t = smallpool.tile([C, NCH], f32, tag="ln", name=f"ln{n}")
            nc.scalar.activation(ln_t[:], ss_ps[:], Act.Ln)
            lninv = smallpool.tile([C, NCH], f32, tag="lninv", name=f"lninv{n}")
            nc.vector.tensor_scalar_mul(lninv[:], ln_t[:], -0.5)
            inv_n = smallpool.tile([C, NCH], f32, tag="invn", name=f"invn{n}")
            nc.scalar.activation(inv_n[:], lninv[:], Act.Exp)
            nrm_bf = smallpool.tile([C, NCH], bf16, tag="nrm", name=f"nrm{n}")
            nc.scalar.activation(nrm_bf[:], ln_t[:], Act.Exp, scale=0.5)

            # --- per-chunk: transpose x, logits^T, exp ---
            # xT layout: (128, NCH, 132): cols 0..127 = x^T chunk, col 128 = n[l]
            # (col 128 recovers Z: sum_l (e*inv_n)[k,l] * n[l] = sum_l e[k,l])
            xt_sb = xtpool.tile([C, NCH, 132], bf16, tag="xt")
            nc.vector.tensor_copy(xt_sb[:, :, 128], nrm_bf[:])
            # E'^T layout: (128, NCH, 66): cols 0..63 = exp chunk, col 64 = ones
            et_sb = etpool.tile([C, NCH, 66], bf16, tag="et")
            nc.vector.memset(et_sb[:, :, 64:65], 1.0)

            GX = 4   # xT chunks per psum group (1 bank)
            GL = 8   # logitsT chunks per psum group (1 bank)
            for g in range(NCH // GX):
                xt_ps = ps_xt.tile([C, GX * 128], bf16, tag="xt_ps")
                for jj in range(GX):
                    j = g * GX + jj
                    nc.tensor.transpose(
                        xt_ps[:, jj * 128:(jj + 1) * 128],
                        x_bf[:, j * 128:(j + 1) * 128],
                        idn_sb[:],
                    )
                # PSUM(bf16) -> SBUF(bf16) copy, 2x mode
                nc.vector.tensor_copy(
                    xt_sb[:, g * GX:(g + 1) * GX, 0:128], xt_ps[:]
                )
            for g in range(NCH // GL):
                lg_ps = ps_lg.tile([C, GL * K], f32, tag="lg_ps")
                for jj in range(GL):
                    j = g * GL + jj
                    nc.tensor.matmul(
                        lg_ps[:, jj * K:(jj + 1) * K],
                        lhsT=x_bf[:, j * 128:(j + 1) * 128],
                        rhs=mt_sb[:],
                        start=True, stop=True,
                    )
                for jj in range(GL):
                    j = g * GL + jj
                    # E'^T = exp(a*inv_n - 0.5 ln ss) = exp(a*inv_n) * inv_n
                    nc.scalar.activation(
                        et_sb[:, j, 0:K],
                        lg_ps[:, jj * K:(jj + 1) * K],
                        Act.Exp,
                        scale=col(inv_n, j),
                        bias=col(lninv, j),
                    )

            # --- A^T and Z via NACC parallel accumulated matmul chains ---
            # out (65,129): rows 0..63 = A(k,c) cols 0..127, col 128 = Z[k]
            az_parts = [
                ps_az.tile([65, 129], f32, tag=f"az{a}", name=f"az{a}_{n}")
                for a in range(NACC)
            ]
            SPAN = NCH // NACC
            for a in range(NACC):
                for jj in range(SPAN):
                    j = a * SPAN + jj
                    nc.tensor.matmul(
                        az_parts[a][:],
                        lhsT=et_sb[:, j, 0:65],
                        rhs=xt_sb[:, j, 0:129],
                        start=(jj == 0), stop=(jj == SPAN - 1),
                    )
            # combine the partial accumulators (one PSUM operand per DVE op,
            # and never in-place)
            prev = None
            for a in range(NACC):
                nxt = smallpool.tile([65, 129], f32, tag=f"azc{a}", name=f"azc{a}_{n}")
                if prev is None:
                    nc.vector.tensor_copy(nxt[:], az_parts[a][:])
                else:
                    nc.vector.tensor_add(nxt[:], az_parts[a][:], prev[:])
                prev = nxt
            az_sb = prev

            # --- finalize ---
            rz = smallpool.tile([K, 1], f32, tag="rz")
            nc.vector.reciprocal(rz[:], az_sb[0:K, 128:129])
            at_ps = ps_fin.tile([C, K], f32, tag="fin")
            nc.tensor.transpose(at_ps[:], az_sb[0:K, 0:128], id32_sb[:])
            at_sb = smallpool.tile([C, K], f32r, tag="at_sb")
            nc.vector.tensor_copy(at_sb[:], at_ps[:])
            vl_ps = ps_fin.tile([K, D], f32, tag="fin")
            nc.tensor.matmul(
                vl_ps[:],
                lhsT=at_sb[:],
                rhs=wt_sb[:],
                start=True, stop=True,
            )
            vl_sb = outpool.tile([K, D], f32, tag="vl_sb")
            nc.vector.tensor_scalar(
                out=vl_sb[:], in0=vl_ps[:], scalar1=rz[:], scalar2=None, op0=Alu.mult
            )
            nc.sync.dma_start(out_d[n], vl_sb[:])

    nc.compile()
    return nc


def _get_program():
    if "nc" not in _CACHE:
        _CACHE["nc"] = _build_program()
    return _CACHE["nc"]


def _host_prep(conv_w, conv_b, centroids):
    import ml_dtypes

    cn = centroids / np.maximum(
        np.sqrt((centroids * centroids).sum(1, keepdims=True)), 1e-12
    )
    m = cn @ conv_w                                   # (K, C)
    mt = np.ascontiguousarray(m.T).astype(ml_dtypes.bfloat16)      # (C, K)
    wt = np.ascontiguousarray(conv_w.T).astype(np.float32)         # (C, D)
    idn = np.eye(C, dtype=ml_dtypes.bfloat16)
    id32 = np.eye(K, dtype=np.float32)
    return mt, wt, idn, id32


def _install_ntff_hook():
    """The image's antenv package lacks axon_hooks, so boot() skipped NTFF
    profiling setup. Recreate the module + install the ctypes hook so
    trace=True yields per-core exec times."""
    import sys as _sys
    import types as _types

    if "antenv.axon_hooks" in _sys.modules:
        return
    try:
        from trn_agent_boot.trn_boot import _ntff_profile_via_ctypes
        hook = _ntff_profile_via_ctypes("/opt/axon/libaxon_pjrt.so")
    except Exception:
        hook = None
    mod = _types.ModuleType("antenv.axon_hooks")
    mod._hook = hook
    mod.get_axon_ntff_profile_hook = lambda: mod._hook
    def _set(h):
        mod._hook = h
    mod.set_axon_ntff_profile_hook = _set
    _sys.modules["antenv.axon_hooks"] = mod


def kernel(x, conv_w, conv_b, centroids):
    _install_ntff_hook()
    from concourse.bass_utils import run_bass_kernel_spmd

    x = np.asarray(x, dtype=np.float32)
    conv_w = np.asarray(conv_w, dtype=np.float32)
    conv_b = np.asarray(conv_b, dtype=np.float32)
    centroids = np.asarray(centroids, dtype=np.float32)

    nc = _get_program()
    mt, wt, idn, id32 = _host_prep(conv_w, conv_b, centroids)

    xr = x.reshape(N, C, HW)
    in_maps = []
    for c in range(NCORES):
        in_maps.append({
            "x": np.ascontiguousarray(xr[c * NB:(c + 1) * NB]),
            "mt": mt, "wt": wt, "idn": idn, "id32": id32,
        })

    trace = bool(int(os.environ.get("NETVLAD_TRACE", "0")))
    res = run_bass_kernel_spmd(nc, in_maps, core_ids=list(range(NCORES)), trace=trace)
    _CACHE["last_result"] = res

    out = np.concatenate([r["out"] for r in res.results], axis=0)  # (16, K, D)
    if np.any(conv_b):
        out = out + conv_b[None, None, :]
    return out.astype(np.float32)
